# revision 22
# baseline (speedup 1.0000x reference)
"""FSEmbedLoss Trainium2 kernel (8 NeuronCores, SPMD data-parallel over batch).

Math (equivalent restructuring of the reference):
  feats x_p (D=256) per pixel, labels l_p in [0,19).
  sums_c   = sum_{p: l_p=c} x_p
  usums_c  = sum_{p: l_p=c} x_p / |x_p|
  counts_c = #{p: l_p=c}
  chat_c   = sums_c / |sums_c|          (== center_c / |center_c|)
  sim_loss  = sum_{c present} (1 - (usums_c . chat_c) / counts_c)
  diff_loss = sum_{i present} mean_j [ j==i ? 1 - Z_ii : relu(Z_ij) ],  Z = chat chat^T
All three big reductions contract over pixels -> TensorE matmuls with a
[onehot | onehot/|x|] (128, 38) stationary operand against pixel-partitioned
transposed tiles.  Sharding: core k takes batch image k (data-parallel),
partials (38, 257) all-reduced across the 8 cores, epilogue replicated.
"""

import os
import sys

import numpy as np

_REPO = "/opt/trn_rl_repo"
if _REPO not in sys.path:
    sys.path.insert(0, _REPO)

B, D, H, W = 8, 256, 128, 128
C = 19
NCORES = 8
PX = H * W  # 16384 pixels per core (one batch image)
P = 128  # partitions / matmul contraction size
SG_PX = int(os.environ.get("FSE_SG_PX", "2048"))  # pixels per DMA supergroup
G_PX = 512  # pixels per norm/copy group
N_SG = PX // SG_PX  # 8
N_G = PX // G_PX  # 32
KSUB = G_PX // P  # 4 x 128-px subgroups per group
M = 2 * C  # 38 stationary columns: [onehot | onehot/|x|]
NW = D + 1  # 257 moving columns: [x_T | ones]


def _build_module(f32_load=None):
    import concourse.bass as bass
    import concourse.tile as tile
    from concourse import bacc
    from concourse import mybir
    from concourse.masks import make_identity

    if f32_load is None:
        f32_load = bool(os.environ.get("FSE_F32_LOAD"))

    f32 = mybir.dt.float32
    bf16 = mybir.dt.bfloat16
    i32 = mybir.dt.int32

    nc = bacc.Bacc(num_devices=NCORES)

    x_dram = nc.dram_tensor("x", [D, PX], f32, kind="ExternalInput")
    lab_dram = nc.dram_tensor("labels", [PX], i32, kind="ExternalInput")
    out_dram = nc.dram_tensor("out", [1], f32, kind="ExternalOutput")
    partial_dram = nc.dram_tensor("partial", [M, NW], f32)
    reduced_dram = nc.dram_tensor("reduced", [M, NW], f32, addr_space="Shared")

    def bc_inner(ap, n):
        # append a zero-stride (broadcast) innermost dim
        return bass.AP(tensor=ap.tensor, offset=ap.offset, ap=[*ap.ap, [0, n]])

    def bc_mid(ap, n):
        # insert a zero-stride (broadcast) dim between partition and free
        return bass.AP(
            tensor=ap.tensor, offset=ap.offset, ap=[ap.ap[0], [0, n], *ap.ap[1:]]
        )

    with tile.TileContext(nc) as tc:
        from contextlib import ExitStack

        with ExitStack() as ctx:
            consts = ctx.enter_context(tc.tile_pool(name="consts", bufs=1))
            nat = ctx.enter_context(tc.tile_pool(name="nat", bufs=3))
            ohp = ctx.enter_context(tc.tile_pool(name="ohp", bufs=2))
            small = ctx.enter_context(tc.tile_pool(name="small", bufs=4))
            stat = ctx.enter_context(tc.tile_pool(name="stat", bufs=1))
            pT = ctx.enter_context(tc.tile_pool(name="pT", bufs=3, space="PSUM"))
            pAcc = ctx.enter_context(tc.tile_pool(name="pAcc", bufs=1, space="PSUM"))
            pFin = ctx.enter_context(tc.tile_pool(name="pFin", bufs=1, space="PSUM"))

            # ---------------- constants ----------------
            ident_bf = consts.tile([P, P], bf16)
            make_identity(nc, ident_bf)
            ident_f32 = consts.tile([P, P], f32)
            make_identity(nc, ident_f32)
            iota_i = consts.tile([P, C], i32)
            nc.gpsimd.iota(iota_i, pattern=[[1, C]], base=0, channel_multiplier=0)
            iota_f = consts.tile([P, C], f32)
            nc.vector.tensor_copy(out=iota_f, in_=iota_i)
            eye19 = consts.tile([C, C], f32)
            make_identity(nc, eye19)
            ones19 = consts.tile([C, 1], f32)
            nc.vector.memset(ones19, 1.0)

            # ---------------- labels -> pixel-partitioned fp32 ----------------
            lab_i = stat.tile([P, PX // P], i32)
            nc.sync.dma_start(
                out=lab_i, in_=lab_dram[:].rearrange("(p t) -> p t", p=P)
            )
            lab_f = stat.tile([P, PX // P], f32)
            nc.vector.tensor_copy(out=lab_f, in_=lab_i)
            lab_ps = pFin.tile([P, P], f32, tag="fin")
            nc.tensor.transpose(lab_ps, lab_f, ident_f32)
            labT = stat.tile([P, PX // P], f32)  # [:, g] = labels of group g
            nc.scalar.copy(out=labT, in_=lab_ps)

            # ---------------- stable transposed-x tiles (double buffer) -------
            # layout (P, KSUB, NW): per 128-px subgroup k, cols 0..255 = x_T,
            # col 256 = ones (written once, never overwritten)
            xt = []
            for i in range(4):
                t = stat.tile([P, KSUB, NW], bf16, tag=f"xt{i}")
                nc.vector.memset(t[:, :, D : D + 1], 1.0)
                xt.append(t)

            # psum accumulator for [onehot|phi]^T @ [xT|1]
            acc_ps = pAcc.tile([M, NW], f32)

            # dead store target for the fused square+accumulate norm ops
            sq_scratch = stat.tile([P, D], bf16)

            # ---------------- main loop ----------------
            dma_only = bool(os.environ.get("FSE_DMA_ONLY"))
            nat_dt = f32 if f32_load else bf16
            t_ident = ident_f32 if f32_load else ident_bf
            for sg in range(N_SG):
                natx = []
                for c in range(2):
                    nt = nat.tile([P, SG_PX], nat_dt, tag=f"nat{c}")
                    src = x_dram[c * P : (c + 1) * P, sg * SG_PX : (sg + 1) * SG_PX]
                    if f32_load:
                        nc.sync.dma_start(out=nt, in_=src)
                    else:
                        nc.gpsimd.dma_start(out=nt, in_=src)
                    natx.append(nt)
                if dma_only:
                    continue

                # onehot for the 16 groups of this supergroup: (P, 16, 38)
                n_gl = SG_PX // P  # 16 column-groups in labT
                oh = ohp.tile([P, SG_PX // P, M], bf16, tag="oh")
                nc.vector.tensor_tensor(
                    out=oh[:, :, 0:C],
                    in0=bc_inner(labT[:, sg * n_gl : (sg + 1) * n_gl], C),
                    in1=bc_mid(iota_f[:, 0:C], n_gl),
                    op=mybir.AluOpType.is_equal,
                )

                n_g4 = SG_PX // G_PX  # 4 groups of 512 px
                # stage A: transpose -> {copy || fused square+accum}
                nrm2 = small.tile([P, n_g4 * KSUB], f32, tag="nrm2")
                for g4 in range(n_g4):
                    ps = pT.tile([P, KSUB * D], nat_dt, tag="pT")
                    for k in range(KSUB):
                        for c in range(2):
                            px0 = g4 * G_PX + k * P
                            nc.tensor.transpose(
                                ps[:, k * D + c * P : k * D + c * P + P],
                                natx[c][:, px0 : px0 + P],
                                t_ident,
                            )
                    nc.vector.tensor_copy(
                        out=xt[g4][:, :, 0:D],
                        in_=ps[:].rearrange("p (k d) -> p k d", k=KSUB),
                    )
                    for k in range(KSUB):
                        nc.scalar.activation(
                            out=sq_scratch,
                            in_=ps[:, k * D : (k + 1) * D],
                            func=mybir.ActivationFunctionType.Square,
                            accum_out=nrm2[:, g4 * KSUB + k : g4 * KSUB + k + 1],
                        )
                # supergroup-batched |x| and 1/|x|
                nrm = small.tile([P, n_g4 * KSUB], f32, tag="nrm")
                nc.scalar.activation(
                    out=nrm, in_=nrm2, func=mybir.ActivationFunctionType.Sqrt
                )
                rinv = small.tile([P, n_g4 * KSUB], f32, tag="rinv")
                nc.vector.reciprocal(out=rinv, in_=nrm)
                # stage B: phi (one op for the whole supergroup) + matmuls
                nc.vector.tensor_tensor(
                    out=oh[:, :, C:M],
                    in0=oh[:, :, 0:C],
                    in1=bc_inner(rinv[:, 0 : n_g4 * KSUB], C),
                    op=mybir.AluOpType.mult,
                )
                for g4 in range(n_g4):
                    g = sg * n_g4 + g4
                    for k in range(KSUB):
                        nc.tensor.matmul(
                            acc_ps,
                            lhsT=oh[:, g4 * KSUB + k, :],
                            rhs=xt[g4][:, k, :],
                            start=(g == 0 and k == 0),
                            stop=(g == N_G - 1 and k == KSUB - 1),
                        )

            # ---------------- all-reduce partials ----------------
            acc_sb = stat.tile([M, NW], f32)
            nc.scalar.copy(out=acc_sb, in_=acc_ps)
            nc.sync.dma_start(out=partial_dram[:, :], in_=acc_sb)
            if os.environ.get("FSE_NO_COLLECTIVE"):
                nc.gpsimd.dma_start(out=reduced_dram[:, :], in_=partial_dram[:, :])
            else:
                nc.gpsimd.collective_compute(
                    "AllReduce",
                    mybir.AluOpType.add,
                    replica_groups=[list(range(NCORES))],
                    ins=[partial_dram[:, :]],
                    outs=[reduced_dram[:, :]],
                )
            Rs = stat.tile([C, NW], f32, tag="Rs")
            nc.sync.dma_start(out=Rs, in_=reduced_dram[0:C, :])
            Ru = stat.tile([C, NW], f32, tag="Ru")
            nc.sync.dma_start(out=Ru, in_=reduced_dram[C:M, :])

            # ---------------- tiny replicated epilogue ----------------
            sums = Rs[0:C, 0:D]
            usums = Ru[0:C, 0:D]
            counts = Rs[0:C, D : D + 1]

            sc1 = small.tile([C, 1], f32, tag="sc1")  # |sums|^2 -> |sums|
            ep_scr = stat.tile([C, D], f32)
            nc.scalar.square(out=ep_scr, in_=sums)
            nc.vector.tensor_reduce(
                out=sc1, in_=ep_scr, axis=mybir.AxisListType.X, op=mybir.AluOpType.add
            )
            # clamp to avoid 0-norm (absent classes) -> inf/nan
            nc.vector.tensor_scalar_max(out=sc1, in0=sc1, scalar1=1e-12)
            nc.scalar.activation(
                out=sc1, in_=sc1, func=mybir.ActivationFunctionType.Sqrt
            )
            crinv = small.tile([C, 1], f32, tag="crinv")
            nc.vector.reciprocal(out=crinv, in_=sc1)
            chat = stat.tile([C, D], f32)
            nc.vector.tensor_scalar_mul(out=chat, in0=sums, scalar1=crinv)

            # w_c = usums_c . chat_c
            wvec = small.tile([C, 1], f32, tag="wvec")
            nc.vector.tensor_mul(out=ep_scr, in0=usums, in1=chat)
            nc.vector.tensor_reduce(
                out=wvec, in_=ep_scr, axis=mybir.AxisListType.X, op=mybir.AluOpType.add
            )
            cnt1 = small.tile([C, 1], f32, tag="cnt1")
            nc.vector.tensor_scalar_max(out=cnt1, in0=counts, scalar1=1.0)
            rcnt = small.tile([C, 1], f32, tag="rcnt")
            nc.vector.reciprocal(out=rcnt, in_=cnt1)
            # simv = 1 - w * rcnt  (computed as (w*rcnt)*(-1) + 1)
            simv = small.tile([C, 1], f32, tag="simv")
            nc.vector.tensor_scalar(
                out=simv,
                in0=wvec,
                scalar1=rcnt,
                scalar2=-1.0,
                op0=mybir.AluOpType.mult,
                op1=mybir.AluOpType.mult,
            )
            nc.vector.tensor_scalar_add(out=simv, in0=simv, scalar1=1.0)

            # Z = chat chat^T via PE (transpose chat chunks, then matmul)
            chT_ps = pFin.tile([P, 2, C], f32, tag="fin")
            for c in range(2):
                nc.tensor.transpose(
                    chT_ps[:, c, :], chat[:, c * P : (c + 1) * P], ident_f32[0:C, 0:C]
                )
            chT = stat.tile([P, 2, C], f32)
            nc.scalar.copy(out=chT, in_=chT_ps)
            z_ps = pFin.tile([C, C], f32, tag="fin")
            for c in range(2):
                nc.tensor.matmul(
                    z_ps,
                    lhsT=chT[:, c, :],
                    rhs=chT[:, c, :],
                    start=(c == 0),
                    stop=(c == 1),
                )
            zr = stat.tile([C, C], f32)  # relu(Z)
            nc.scalar.activation(
                out=zr, in_=z_ps, func=mybir.ActivationFunctionType.Relu
            )
            # per_pair = relu(Z) + eye*(1 - Z - relu(Z))
            t1 = stat.tile([C, C], f32)
            nc.vector.tensor_add(out=t1, in0=z_ps, in1=zr)  # Z + relu(Z)
            nc.vector.tensor_scalar(
                out=t1,
                in0=t1,
                scalar1=-1.0,
                scalar2=1.0,
                op0=mybir.AluOpType.mult,
                op1=mybir.AluOpType.add,
            )  # 1 - Z - relu(Z)
            nc.vector.tensor_mul(out=t1, in0=t1, in1=eye19)
            nc.vector.tensor_add(out=t1, in0=t1, in1=zr)
            peri = small.tile([C, 1], f32, tag="peri")
            nc.vector.tensor_reduce(
                out=peri, in_=t1, axis=mybir.AxisListType.X, op=mybir.AluOpType.add
            )
            # total_vec = present * (simv + peri/19)
            pres = small.tile([C, 1], f32, tag="pres")
            nc.vector.tensor_single_scalar(
                out=pres, in_=counts, scalar=0.0, op=mybir.AluOpType.is_gt
            )
            tot = small.tile([C, 1], f32, tag="tot")
            nc.vector.tensor_scalar(
                out=tot,
                in0=peri,
                scalar1=1.0 / C,
                scalar2=simv,
                op0=mybir.AluOpType.mult,
                op1=mybir.AluOpType.add,
            )
            nc.vector.tensor_mul(out=tot, in0=tot, in1=pres)
            # sum over the 19 partitions via matmul with ones
            res_ps = pFin.tile([1, 1], f32, tag="fin")
            nc.tensor.matmul(res_ps, lhsT=tot, rhs=ones19, start=True, stop=True)
            res_sb = small.tile([1, 1], f32, tag="res_sb")
            nc.scalar.copy(out=res_sb, in_=res_ps)
            nc.sync.dma_start(out=out_dram[0:1], in_=res_sb)

    return nc


_NC_CACHE = {}


def _get_nc(f32_load=None):
    key = ("nc", f32_load)
    if key not in _NC_CACHE:
        nc = _build_module(f32_load=f32_load)
        nc.finalize()
        _NC_CACHE[key] = nc
    return _NC_CACHE[key]


def _make_in_maps(inputs, targets):
    inputs = np.asarray(inputs, dtype=np.float32)
    targets = np.asarray(targets)
    if targets.dtype != np.int32:
        targets = targets.astype(np.int32)
    in_maps = []
    for k in range(NCORES):
        in_maps.append(
            {
                "x": np.ascontiguousarray(inputs[k].reshape(D, PX)),
                "labels": np.ascontiguousarray(targets[k].reshape(PX)),
            }
        )
    return in_maps


def _run(inputs, targets, trace=False, tmpdir=None):
    from concourse.bass_utils import run_bass_kernel_spmd

    nc = _get_nc()
    in_maps = _make_in_maps(inputs, targets)
    r = run_bass_kernel_spmd(
        nc, in_maps, list(range(NCORES)), trace=trace, tmpdir=tmpdir
    )
    out = np.asarray(r.results[0]["out"], dtype=np.float32).reshape(1)
    return out, r


def kernel(inputs, targets):
    out, _ = _run(inputs, targets, trace=False)
    return out


# revision 23
# speedup vs baseline: 1.1183x; 1.1183x over previous
"""FSEmbedLoss Trainium2 kernel (8 NeuronCores, SPMD data-parallel over batch).

Math (equivalent restructuring of the reference):
  feats x_p (D=256) per pixel, labels l_p in [0,19).
  sums_c   = sum_{p: l_p=c} x_p
  usums_c  = sum_{p: l_p=c} x_p / |x_p|
  counts_c = #{p: l_p=c}
  chat_c   = sums_c / |sums_c|          (== center_c / |center_c|)
  sim_loss  = sum_{c present} (1 - (usums_c . chat_c) / counts_c)
  diff_loss = sum_{i present} mean_j [ j==i ? 1 - Z_ii : relu(Z_ij) ],  Z = chat chat^T
All three big reductions contract over pixels -> TensorE matmuls with a
[onehot | onehot/|x|] (128, 38) stationary operand against pixel-partitioned
transposed tiles.  Sharding: core k takes batch image k (data-parallel),
partials (38, 257) all-reduced across the 8 cores, epilogue replicated.
"""

import os
import sys

import numpy as np

_REPO = "/opt/trn_rl_repo"
if _REPO not in sys.path:
    sys.path.insert(0, _REPO)

B, D, H, W = 8, 256, 128, 128
C = 19
NCORES = 8
PX = H * W  # 16384 pixels per core (one batch image)
P = 128  # partitions / matmul contraction size
SG_PX = int(os.environ.get("FSE_SG_PX", "2048"))  # pixels per DMA supergroup
G_PX = 512  # pixels per norm/copy group
N_SG = PX // SG_PX  # 8
N_G = PX // G_PX  # 32
KSUB = G_PX // P  # 4 x 128-px subgroups per group
M = 2 * C  # 38 stationary columns: [onehot | onehot/|x|]
NW = D + 1  # 257 moving columns: [x_T | ones]


def _build_module(f32_load=None):
    import concourse.bass as bass
    import concourse.tile as tile
    from concourse import bacc
    from concourse import mybir
    from concourse.masks import make_identity

    if f32_load is None:
        f32_load = bool(os.environ.get("FSE_F32_LOAD"))

    f32 = mybir.dt.float32
    bf16 = mybir.dt.bfloat16
    i32 = mybir.dt.int32

    nc = bacc.Bacc(num_devices=NCORES)

    x_dram = nc.dram_tensor("x", [D, PX], f32, kind="ExternalInput")
    lab_dram = nc.dram_tensor("labels", [PX], i32, kind="ExternalInput")
    out_dram = nc.dram_tensor("out", [1], f32, kind="ExternalOutput")
    partial_dram = nc.dram_tensor("partial", [M, NW], f32)
    reduced_dram = nc.dram_tensor("reduced", [M, NW], f32, addr_space="Shared")

    def bc_inner(ap, n):
        # append a zero-stride (broadcast) innermost dim
        return bass.AP(tensor=ap.tensor, offset=ap.offset, ap=[*ap.ap, [0, n]])

    def bc_mid(ap, n):
        # insert a zero-stride (broadcast) dim between partition and free
        return bass.AP(
            tensor=ap.tensor, offset=ap.offset, ap=[ap.ap[0], [0, n], *ap.ap[1:]]
        )

    with tile.TileContext(nc) as tc:
        from contextlib import ExitStack

        with ExitStack() as ctx:
            consts = ctx.enter_context(tc.tile_pool(name="consts", bufs=1))
            nat = ctx.enter_context(tc.tile_pool(name="nat", bufs=4))
            ohp = ctx.enter_context(tc.tile_pool(name="ohp", bufs=2))
            small = ctx.enter_context(tc.tile_pool(name="small", bufs=4))
            stat = ctx.enter_context(tc.tile_pool(name="stat", bufs=1))
            pT = ctx.enter_context(tc.tile_pool(name="pT", bufs=3, space="PSUM"))
            pAcc = ctx.enter_context(tc.tile_pool(name="pAcc", bufs=1, space="PSUM"))
            pFin = ctx.enter_context(tc.tile_pool(name="pFin", bufs=1, space="PSUM"))

            # ---------------- constants ----------------
            ident_bf = consts.tile([P, P], bf16)
            make_identity(nc, ident_bf)
            ident_f32 = consts.tile([P, P], f32)
            make_identity(nc, ident_f32)
            iota_i = consts.tile([P, C], i32)
            nc.gpsimd.iota(iota_i, pattern=[[1, C]], base=0, channel_multiplier=0)
            iota_f = consts.tile([P, C], f32)
            nc.vector.tensor_copy(out=iota_f, in_=iota_i)
            eye19 = consts.tile([C, C], f32)
            make_identity(nc, eye19)
            ones19 = consts.tile([C, 1], f32)
            nc.vector.memset(ones19, 1.0)

            # ---------------- labels -> pixel-partitioned fp32 ----------------
            lab_i = stat.tile([P, PX // P], i32)
            nc.sync.dma_start(
                out=lab_i, in_=lab_dram[:].rearrange("(p t) -> p t", p=P)
            )
            lab_f = stat.tile([P, PX // P], f32)
            nc.vector.tensor_copy(out=lab_f, in_=lab_i)
            lab_ps = pFin.tile([P, P], f32, tag="fin")
            nc.tensor.transpose(lab_ps, lab_f, ident_f32)
            labT = stat.tile([P, PX // P], f32)  # [:, g] = labels of group g
            nc.scalar.copy(out=labT, in_=lab_ps)

            # ---------------- stable transposed-x tiles (double buffer) -------
            # layout (P, KSUB, NW): per 128-px subgroup k, cols 0..255 = x_T,
            # col 256 = ones (written once, never overwritten)
            xt = []
            for i in range(4):
                t = stat.tile([P, KSUB, NW], bf16, tag=f"xt{i}")
                nc.vector.memset(t[:, :, D : D + 1], 1.0)
                xt.append(t)

            # PE HAM warm-up: ~40 back-to-back matmuls (~4.3us cold) during
            # the initial DMA fill window flip the PE clock gate to 2.4 GHz
            # before the first real transposes (transpose-mode alone never
            # warms the HAM).
            warm_ps = pFin.tile([P, P], f32, tag="warm")
            for i in range(40):
                nc.tensor.matmul(
                    warm_ps,
                    lhsT=ident_bf,
                    rhs=ident_bf,
                    start=(i == 0),
                    stop=(i == 39),
                )

            # psum accumulator for [onehot|phi]^T @ [xT|1]
            acc_ps = pAcc.tile([M, NW], f32)

            # dead store target for the fused square+accumulate norm ops
            sq_scratch = stat.tile([P, D], bf16)

            # ---------------- main loop ----------------
            dma_only = bool(os.environ.get("FSE_DMA_ONLY"))
            nat_dt = f32 if f32_load else bf16
            t_ident = ident_f32 if f32_load else ident_bf
            for sg in range(N_SG):
                natx = []
                for c in range(2):
                    nt = nat.tile([P, SG_PX], nat_dt, tag=f"nat{c}")
                    src = x_dram[c * P : (c + 1) * P, sg * SG_PX : (sg + 1) * SG_PX]
                    if f32_load:
                        nc.sync.dma_start(out=nt, in_=src)
                    else:
                        nc.gpsimd.dma_start(out=nt, in_=src)
                    natx.append(nt)
                if dma_only:
                    continue

                # onehot for the 16 groups of this supergroup: (P, 16, 38)
                n_gl = SG_PX // P  # 16 column-groups in labT
                oh = ohp.tile([P, SG_PX // P, M], bf16, tag="oh")
                nc.vector.tensor_tensor(
                    out=oh[:, :, 0:C],
                    in0=bc_inner(labT[:, sg * n_gl : (sg + 1) * n_gl], C),
                    in1=bc_mid(iota_f[:, 0:C], n_gl),
                    op=mybir.AluOpType.is_equal,
                )

                n_g4 = SG_PX // G_PX  # 4 groups of 512 px
                # stage A: transpose -> {copy || fused square+accum}
                nrm2 = small.tile([P, n_g4 * KSUB], f32, tag="nrm2")
                for g4 in range(n_g4):
                    ps = pT.tile([P, KSUB * D], nat_dt, tag="pT")
                    for k in range(KSUB):
                        for c in range(2):
                            px0 = g4 * G_PX + k * P
                            nc.tensor.transpose(
                                ps[:, k * D + c * P : k * D + c * P + P],
                                natx[c][:, px0 : px0 + P],
                                t_ident,
                            )
                    nc.vector.tensor_copy(
                        out=xt[g4][:, :, 0:D],
                        in_=ps[:].rearrange("p (k d) -> p k d", k=KSUB),
                    )
                    for k in range(KSUB):
                        nc.scalar.activation(
                            out=sq_scratch,
                            in_=ps[:, k * D : (k + 1) * D],
                            func=mybir.ActivationFunctionType.Square,
                            accum_out=nrm2[:, g4 * KSUB + k : g4 * KSUB + k + 1],
                        )
                # supergroup-batched |x| and 1/|x|
                nrm = small.tile([P, n_g4 * KSUB], f32, tag="nrm")
                nc.scalar.activation(
                    out=nrm, in_=nrm2, func=mybir.ActivationFunctionType.Sqrt
                )
                rinv = small.tile([P, n_g4 * KSUB], f32, tag="rinv")
                nc.vector.reciprocal(out=rinv, in_=nrm)
                # stage B: phi (one op for the whole supergroup) + matmuls
                nc.vector.tensor_tensor(
                    out=oh[:, :, C:M],
                    in0=oh[:, :, 0:C],
                    in1=bc_inner(rinv[:, 0 : n_g4 * KSUB], C),
                    op=mybir.AluOpType.mult,
                )
                for g4 in range(n_g4):
                    g = sg * n_g4 + g4
                    for k in range(KSUB):
                        nc.tensor.matmul(
                            acc_ps,
                            lhsT=oh[:, g4 * KSUB + k, :],
                            rhs=xt[g4][:, k, :],
                            start=(g == 0 and k == 0),
                            stop=(g == N_G - 1 and k == KSUB - 1),
                        )

            # ---------------- all-reduce partials ----------------
            acc_sb = stat.tile([M, NW], f32)
            nc.scalar.copy(out=acc_sb, in_=acc_ps)
            nc.sync.dma_start(out=partial_dram[:, :], in_=acc_sb)
            if os.environ.get("FSE_NO_COLLECTIVE"):
                nc.gpsimd.dma_start(out=reduced_dram[:, :], in_=partial_dram[:, :])
            else:
                nc.gpsimd.collective_compute(
                    "AllReduce",
                    mybir.AluOpType.add,
                    replica_groups=[list(range(NCORES))],
                    ins=[partial_dram[:, :]],
                    outs=[reduced_dram[:, :]],
                )
            Rs = stat.tile([C, NW], f32, tag="Rs")
            nc.sync.dma_start(out=Rs, in_=reduced_dram[0:C, :])
            Ru = stat.tile([C, NW], f32, tag="Ru")
            nc.sync.dma_start(out=Ru, in_=reduced_dram[C:M, :])

            # ---------------- tiny replicated epilogue ----------------
            sums = Rs[0:C, 0:D]
            usums = Ru[0:C, 0:D]
            counts = Rs[0:C, D : D + 1]

            sc1 = small.tile([C, 1], f32, tag="sc1")  # |sums|^2 -> |sums|
            ep_scr = stat.tile([C, D], f32)
            nc.scalar.square(out=ep_scr, in_=sums)
            nc.vector.tensor_reduce(
                out=sc1, in_=ep_scr, axis=mybir.AxisListType.X, op=mybir.AluOpType.add
            )
            # clamp to avoid 0-norm (absent classes) -> inf/nan
            nc.vector.tensor_scalar_max(out=sc1, in0=sc1, scalar1=1e-12)
            nc.scalar.activation(
                out=sc1, in_=sc1, func=mybir.ActivationFunctionType.Sqrt
            )
            crinv = small.tile([C, 1], f32, tag="crinv")
            nc.vector.reciprocal(out=crinv, in_=sc1)
            chat = stat.tile([C, D], f32)
            nc.vector.tensor_scalar_mul(out=chat, in0=sums, scalar1=crinv)

            # w_c = usums_c . chat_c
            wvec = small.tile([C, 1], f32, tag="wvec")
            nc.vector.tensor_mul(out=ep_scr, in0=usums, in1=chat)
            nc.vector.tensor_reduce(
                out=wvec, in_=ep_scr, axis=mybir.AxisListType.X, op=mybir.AluOpType.add
            )
            cnt1 = small.tile([C, 1], f32, tag="cnt1")
            nc.vector.tensor_scalar_max(out=cnt1, in0=counts, scalar1=1.0)
            rcnt = small.tile([C, 1], f32, tag="rcnt")
            nc.vector.reciprocal(out=rcnt, in_=cnt1)
            # simv = 1 - w * rcnt  (computed as (w*rcnt)*(-1) + 1)
            simv = small.tile([C, 1], f32, tag="simv")
            nc.vector.tensor_scalar(
                out=simv,
                in0=wvec,
                scalar1=rcnt,
                scalar2=-1.0,
                op0=mybir.AluOpType.mult,
                op1=mybir.AluOpType.mult,
            )
            nc.vector.tensor_scalar_add(out=simv, in0=simv, scalar1=1.0)

            # Z = chat chat^T via PE (transpose chat chunks, then matmul)
            chT_ps = pFin.tile([P, 2, C], f32, tag="fin")
            for c in range(2):
                nc.tensor.transpose(
                    chT_ps[:, c, :], chat[:, c * P : (c + 1) * P], ident_f32[0:C, 0:C]
                )
            chT = stat.tile([P, 2, C], f32)
            nc.scalar.copy(out=chT, in_=chT_ps)
            z_ps = pFin.tile([C, C], f32, tag="fin")
            for c in range(2):
                nc.tensor.matmul(
                    z_ps,
                    lhsT=chT[:, c, :],
                    rhs=chT[:, c, :],
                    start=(c == 0),
                    stop=(c == 1),
                )
            zr = stat.tile([C, C], f32)  # relu(Z)
            nc.scalar.activation(
                out=zr, in_=z_ps, func=mybir.ActivationFunctionType.Relu
            )
            # per_pair = relu(Z) + eye*(1 - Z - relu(Z))
            t1 = stat.tile([C, C], f32)
            nc.vector.tensor_add(out=t1, in0=z_ps, in1=zr)  # Z + relu(Z)
            nc.vector.tensor_scalar(
                out=t1,
                in0=t1,
                scalar1=-1.0,
                scalar2=1.0,
                op0=mybir.AluOpType.mult,
                op1=mybir.AluOpType.add,
            )  # 1 - Z - relu(Z)
            nc.vector.tensor_mul(out=t1, in0=t1, in1=eye19)
            nc.vector.tensor_add(out=t1, in0=t1, in1=zr)
            peri = small.tile([C, 1], f32, tag="peri")
            nc.vector.tensor_reduce(
                out=peri, in_=t1, axis=mybir.AxisListType.X, op=mybir.AluOpType.add
            )
            # total_vec = present * (simv + peri/19)
            pres = small.tile([C, 1], f32, tag="pres")
            nc.vector.tensor_single_scalar(
                out=pres, in_=counts, scalar=0.0, op=mybir.AluOpType.is_gt
            )
            tot = small.tile([C, 1], f32, tag="tot")
            nc.vector.tensor_scalar(
                out=tot,
                in0=peri,
                scalar1=1.0 / C,
                scalar2=simv,
                op0=mybir.AluOpType.mult,
                op1=mybir.AluOpType.add,
            )
            nc.vector.tensor_mul(out=tot, in0=tot, in1=pres)
            # sum over the 19 partitions via matmul with ones
            res_ps = pFin.tile([1, 1], f32, tag="fin")
            nc.tensor.matmul(res_ps, lhsT=tot, rhs=ones19, start=True, stop=True)
            res_sb = small.tile([1, 1], f32, tag="res_sb")
            nc.scalar.copy(out=res_sb, in_=res_ps)
            nc.sync.dma_start(out=out_dram[0:1], in_=res_sb)

    return nc


_NC_CACHE = {}


def _get_nc(f32_load=None):
    key = ("nc", f32_load)
    if key not in _NC_CACHE:
        nc = _build_module(f32_load=f32_load)
        nc.finalize()
        _NC_CACHE[key] = nc
    return _NC_CACHE[key]


def _make_in_maps(inputs, targets):
    inputs = np.asarray(inputs, dtype=np.float32)
    targets = np.asarray(targets)
    if targets.dtype != np.int32:
        targets = targets.astype(np.int32)
    in_maps = []
    for k in range(NCORES):
        in_maps.append(
            {
                "x": np.ascontiguousarray(inputs[k].reshape(D, PX)),
                "labels": np.ascontiguousarray(targets[k].reshape(PX)),
            }
        )
    return in_maps


def _run(inputs, targets, trace=False, tmpdir=None):
    from concourse.bass_utils import run_bass_kernel_spmd

    nc = _get_nc()
    in_maps = _make_in_maps(inputs, targets)
    r = run_bass_kernel_spmd(
        nc, in_maps, list(range(NCORES)), trace=trace, tmpdir=tmpdir
    )
    out = np.asarray(r.results[0]["out"], dtype=np.float32).reshape(1)
    return out, r


def kernel(inputs, targets):
    out, _ = _run(inputs, targets, trace=False)
    return out
